# revision 29
# baseline (speedup 1.0000x reference)
"""Trainium2 Bass kernel for nn_GNN_75118978007185 (gnn_message_passing).

Strategy
--------
Pure data-parallel over the batch dim B=8 -> one batch element per core.

All 14 cbr ops (1x1 conv + BN + ReLU) read channels from only 12 source
slabs (f_nodes[0:2], h_nodes[0:3], p_nodes[0:7]), 10 channels each =
120 distinct input channels <= 128 partitions.  So per spatial chunk we
load ONE [120, N] tile and compute ALL 14 outputs (140 channels) with
two fp32 matmuls against packed weight matrices wa/wb [120, 70].

BN scale is folded into the weights on the host; BN shift + ReLU are
fused into a single ScalarE activation per matmul (out = Relu(psum +
shift)).  The `moveaxis` channel interleaves of the reference (upper/
lower p-node stacks) become column permutations of the packed weights,
so no data movement is needed for them at all.

h1 = relu(h1a)+relu(h1b) and h2 = relu(h2a)+relu(h2b) are the only
cross-matmul ops: one VectorE add per chunk.  Matmuls run as float32r
(full-rate fp32 PE mode, ~1e-4 rel err); half the yb ReLUs run on
VectorE to balance ScalarE/VectorE load.

Output channels are packed so each core writes a single [120, HW] DRAM
tensor with two DMAs per tile (yA rows 0:70, yB rows 20:70).  All eight
input loads are issued up-front on the SP HWDGE ring, stores follow on
the same ring; with deep x/y buffering the SDMA pool runs at ~99%
occupancy -- the kernel sits at the HBM roofline (cost-model estimate
~48us/core vs ~44us of compulsory DRAM traffic).
"""

import numpy as np

import concourse.mybir as mybir
from concourse import bacc
from concourse.bass_utils import run_bass_kernel_spmd
from concourse.tile import TileContext

F32 = mybir.dt.float32
F32R = mybir.dt.float32r
N_CORES = 8
H = W = 128
HW = H * W  # 16384 spatial positions
NT = 2048  # SBUF tile width (spatial)
CHUNK = 512  # PSUM bank width in fp32
EPS = 1e-5

# K-row bases of the 12 source slabs in the packed [120, HW] input
_F0, _F1, _H0, _H1, _H2 = 0, 10, 20, 30, 40
_P = [50 + 10 * k for k in range(7)]


def _rng(base):
    return list(range(base, base + 10))


def _op_table():
    """(half, m_base, W_name, src_rows) for each of the 14 cbr ops.

    half 'a' -> yA (rows of first matmul), 'b' -> yB.
    yA rows: h1a@0, h2a@10, f0@20, f1@30, h0@40, p0@50, p1@60
    yB rows: h1b@0, h2b@10, p2@20, p3@30, p4@40, p5@50, p6@60
    The h-blocks sit at partition 0 of both tiles because the VectorE
    add (h1 = relu(h1a)+relu(h1b), h2 likewise) must start at a
    32-aligned partition.  After the add overwrites h1a/h2a, yout =
    [yA[0:70] | yB[20:70]] with row map h1,h2,f0,f1,h0,p0..p6.
    """
    # upper: channel j of the reference's 40-wide concat = p_nodes[1+(j%4)],
    # channel j//4; lower: j -> p_nodes[5+(j%2)], channel j//2.
    upper = [_P[1 + (j % 4)] + (j // 4) for j in range(40)]
    lower = [_P[5 + (j % 2)] + (j // 2) for j in range(20)]
    return [
        ("a", 0, "comp_u", upper + _rng(_H1)),
        ("a", 10, "comp_l", lower + _rng(_H2)),
        ("a", 20, "bg_f", _rng(_F0) + _rng(_H0)),
        ("a", 30, "comp_full", _rng(_F1) + _rng(_H1) + _rng(_H2)),
        ("a", 40, "bg_h", _rng(_F0) + _rng(_H0) + _rng(_P[0])),
        ("a", 50, "bg_p", _rng(_H0) + _rng(_P[0])),
        ("a", 60, "decomp_half", _rng(_H1) + _rng(_P[1])),
        ("b", 0, "decomp_full", _rng(_F1) + _rng(_H1)),
        ("b", 10, "decomp_full", _rng(_F1) + _rng(_H2)),
        ("b", 20, "decomp_half", _rng(_H1) + _rng(_P[2])),
        ("b", 30, "decomp_half", _rng(_H1) + _rng(_P[3])),
        ("b", 40, "decomp_half", _rng(_H1) + _rng(_P[4])),
        ("b", 50, "decomp_half", _rng(_H2) + _rng(_P[5])),
        ("b", 60, "decomp_half", _rng(_H2) + _rng(_P[6])),
    ]


def build_packed(inputs):
    """Pack conv weights (BN scale folded in) and BN shifts.

    Returns wa, wb [120, 70] (lhsT layout: K on partitions, M on free)
    and sa, sb [70, 1] per-output-channel shifts.
    """
    wa = np.zeros((120, 70), np.float32)
    wb = np.zeros((120, 70), np.float32)
    sa = np.zeros((70, 1), np.float32)
    sb = np.zeros((70, 1), np.float32)
    for half, m, name, src in _op_table():
        Wc = np.asarray(inputs["W_" + name], np.float32)  # [10, cin]
        bn = np.asarray(inputs["bn_" + name], np.float32)  # [4, 10]
        gamma, beta, mean, var = bn
        scale = gamma / np.sqrt(var + EPS)
        shift = beta - mean * scale
        wx, sx = (wa, sa) if half == "a" else (wb, sb)
        wx[np.asarray(src), m : m + 10] = (Wc * scale[:, None]).T
        sx[m : m + 10, 0] = shift
    return wa, wb, sa, sb


def _build_program():
    nc = bacc.Bacc()
    xin = nc.declare_dram_parameter("xin", [120, HW], F32, isOutput=False)
    wpack = nc.declare_dram_parameter("wpack", [120, 142], F32, isOutput=False)
    yout = nc.declare_dram_parameter("yout", [120, HW], F32, isOutput=True)
    RELU = mybir.ActivationFunctionType.Relu

    with TileContext(nc) as tc:
        with (
            tc.tile_pool(name="const", bufs=1) as cpool,
            tc.tile_pool(name="x", bufs=8) as xpool,
            tc.tile_pool(name="y", bufs=6) as ypool,
            tc.tile_pool(name="ps", bufs=2, space="PSUM") as pspool,
        ):
            wp_t = cpool.tile([120, 142], F32R, tag="wp")
            nc.sync.dma_start(out=wp_t[:, :], in_=wpack[:, :].bitcast(F32R))
            wa_t = wp_t[:, 0:70]
            wb_t = wp_t[:, 70:140]
            sa_t = wp_t[0:70, 140:141].bitcast(F32)
            sb_t = wp_t[0:70, 141:142].bitcast(F32)
            # PE instructions (LDWEIGHTS) only accept ONE semaphore wait,
            # so no matmul may need both a DMA-completion wait and a
            # PSUM-release wait.  Strategy:
            #  * this one warm-up matmul absorbs the wpack-DMA wait (its
            #    PSUM slot comes from the "pa" tag stream, so its WAW
            #    shows up later as a benign single PE-sem wait);
            #  * chunk 0 of each tile uses dedicated PSUM tags pa0/pb0,
            #    whose release was observed many chunks ago, so the
            #    chunk-0 matmul carries only the x-DMA wait.
            wu = pspool.tile([1, 1], F32, tag="pa")
            nc.tensor.matmul(
                out=wu[:, :], lhsT=wp_t[:, 0:1].bitcast(F32),
                rhs=wp_t[:, 1:2].bitcast(F32),
                start=True, stop=True,
            )
            # Issue every input load up-front: the SP HWDGE ring runs them
            # back-to-back, so the last tile's data lands ~8us earlier than
            # with per-tile loads interleaved between stores.
            xs = []
            for t in range(HW // NT):
                off = t * NT
                x = xpool.tile([120, NT], F32R, tag="x")
                nc.sync.dma_start(out=x[:, :], in_=xin[:, off : off + NT].bitcast(F32R))
                xs.append(x)
            for t in range(HW // NT):
                off = t * NT
                x = xs[t]
                ya = ypool.tile([70, NT], F32, tag="ya")
                yb = ypool.tile([70, NT], F32, tag="yb")
                for c in range(NT // CHUNK):
                    s = slice(c * CHUNK, (c + 1) * CHUNK)
                    atag, btag = ("pa0", "pb0") if c == 0 else ("pa", "pb")
                    # float32r: same bits as fp32, but the PE streams 1
                    # column/cycle at N>=256 instead of fp32's 1/4 rate.
                    pa = pspool.tile([70, CHUNK], F32, tag=atag)
                    nc.tensor.matmul(
                        out=pa[:, :], lhsT=wa_t, rhs=x[:, s],
                        start=True, stop=True,
                    )
                    nc.scalar.activation(
                        out=ya[:, s], in_=pa[:, :], func=RELU,
                        bias=sa_t, scale=1.0,
                    )
                    pb = pspool.tile([70, CHUNK], F32, tag=btag)
                    nc.tensor.matmul(
                        out=pb[:, :], lhsT=wb_t, rhs=x[:, s],
                        start=True, stop=True,
                    )
                    if c % 2 == 1:
                        # Rebalance: ScalarE is the busiest engine, so odd
                        # chunks compute relu(psum + shift) on VectorE via
                        # tensor_scalar (add shift, then max 0).
                        nc.vector.tensor_scalar(
                            out=yb[:, s], in0=pb[:, :],
                            scalar1=sb_t, scalar2=0.0,
                            op0=mybir.AluOpType.add, op1=mybir.AluOpType.max,
                        )
                    else:
                        nc.scalar.activation(
                            out=yb[:, s], in_=pb[:, :], func=RELU,
                            bias=sb_t, scale=1.0,
                        )
                    # h1 = relu(h1a)+relu(h1b); h2 = relu(h2a)+relu(h2b)
                    nc.vector.tensor_add(
                        out=ya[0:20, s], in0=ya[0:20, s], in1=yb[0:20, s]
                    )
                # Stores also go on the SP ring: all loads were issued
                # first (hoisted above), so the FIFO order is L0..L7 then
                # stores -- and issuing stores from the SP sequencer keeps
                # the ACT sequencer free for compute (a store's semaphore
                # wait would otherwise stall ACT's in-order issue).
                nc.sync.dma_start(out=yout[0:70, off : off + NT], in_=ya[:, :])
                nc.sync.dma_start(out=yout[70:120, off : off + NT], in_=yb[20:70, :])
    nc.compile()
    return nc


_NC_CACHE = None


def _get_program():
    global _NC_CACHE
    if _NC_CACHE is None:
        _NC_CACHE = _build_program()
    return _NC_CACHE


def _run(inputs, trace=False):
    wa, wb, sa, sb = build_packed(inputs)
    wpack = np.zeros((120, 142), np.float32)
    wpack[:, 0:70] = wa
    wpack[:, 70:140] = wb
    wpack[0:70, 140:141] = sa
    wpack[0:70, 141:142] = sb
    p = np.asarray(inputs["p_nodes"], np.float32)
    h = np.asarray(inputs["h_nodes"], np.float32)
    f = np.asarray(inputs["f_nodes"], np.float32)
    in_maps = []
    for b in range(N_CORES):
        xin = np.concatenate(
            [
                f[:, b].reshape(20, HW),
                h[:, b].reshape(30, HW),
                p[:, b].reshape(70, HW),
            ],
            axis=0,
        )
        in_maps.append({"xin": np.ascontiguousarray(xin), "wpack": wpack})
    nc = _get_program()
    out = run_bass_kernel_spmd(
        nc, in_maps, list(range(N_CORES)), trace=trace
    )
    res = out.results
    yout = np.stack([np.asarray(res[b]["yout"]) for b in range(N_CORES)], axis=0)
    return _unshard(yout), out


def _unshard(yout):
    """yout [8, 120, HW] rows: h1,h2,f0,f1,h0,p0..p6 -> (p_new, h_new, f_new)."""
    f_new = yout[:, 20:40].reshape(8, 2, 10, H, W).transpose(1, 0, 2, 3, 4)
    h_rows = np.r_[40:50, 0:10, 10:20]
    h_new = yout[:, h_rows].reshape(8, 3, 10, H, W).transpose(1, 0, 2, 3, 4)
    p_new = yout[:, 50:120].reshape(8, 7, 10, H, W).transpose(1, 0, 2, 3, 4)
    return (p_new, h_new, f_new)


def kernel(**inputs):
    (p_new, h_new, f_new), _ = _run(inputs)
    return (p_new, h_new, f_new)
